# revision 4
# baseline (speedup 1.0000x reference)
"""Trainium2 Bass kernel for nn_Encoder (dense MLP with stochastic ternarization).

y = tanh(x @ (s1*T(w1,n1)) + b1) @ (s2*T(w2,n2)) + b2,  T(w,n) = (w-n>1) - (w-n<-1)

Sharding: tensor-parallel over the 16384 hidden dim across 8 cores. Each core
gets a 2048-wide hidden shard of w1/noise1/s1/b1 (column-sharded) and the
matching 2048-row shard of w2/noise2; x is replicated (host pre-transposed to
bf16, tiled per 512-batch block).

v2 (this file) vs the 648us baseline:
- w/n supply is host-packed into contiguous 0.5 MiB ternarize blocks in exact
  consumption order (two 128-wide stripes first so the first m-chains unblock
  ~15us earlier, then 256-wide stripes). w blocks issue on the sync queue,
  n blocks on the gpsimd queue; staging is 4-deep.
- s2/b2 are applied at PSUM evacuation (s2 scale on every core's partial, b2
  added on core 0 only), partials stored bf16, and each (block, quarter) of
  yT goes through its own small bf16 ReduceScatter(add) directly into the
  ExternalOutput tensor. No post-collective compute at all; the last RS is
  256 KiB instead of 1 MiB fp32, cutting the serial tail.
- L1 consumption is per-m-tile interleaved across batch blocks 0/1 so PE
  consumption tracks the DMA-paced ternarize supply.

Ternarization: q = w - noise (DVE), tanh(2^30*(q-1)) + tanh(2^30*(q+1)) (ACT)
== (q>1)-(q<-1) doubled; the factor 2 is folded into s1/s2 on the host.
"""

import sys

for _p in ("/opt/trn_rl_repo",):
    if _p not in sys.path:
        sys.path.insert(0, _p)

import numpy as np
import ml_dtypes

import concourse.bass as bass
import concourse.bacc as bacc
import concourse.mybir as mybir
import concourse.tile as tile
from concourse.bass_utils import run_bass_kernel_spmd

BF16 = mybir.dt.bfloat16
F32 = mybir.dt.float32
FP8 = mybir.dt.float8e4
NPBF16 = ml_dtypes.bfloat16

N_CORES = 8
B = 2048
DIN = 3072
DHID = 16384
DOUT = 1024
HSH = DHID // N_CORES   # 2048
DSH = DOUT // N_CORES   # 128

K1 = DIN // 128          # 24 contraction tiles, layer 1
KG1 = K1 // 4            # 6 groups of 4 k-tiles
K2 = HSH // 128          # 16 contraction tiles, layer 2
KG2 = K2 // 4            # 4 groups of 4 k2-tiles
NB = B // 512            # 4 batch blocks
MT = HSH // 128          # 16 hidden m-tiles
ND = DOUT // 128         # 8 dout tiles
NQ = 4                   # output quarters per block (2 d-tiles each)
QROWS = DOUT // NQ       # 256 dout rows per quarter
QCHUNK = QROWS // N_CORES  # 32 rows per core per quarter

# L1 supply stripes: (col0, width); first two are 128-wide for fast start
STRIPES = [(0, 128), (128, 128)] + [(256 + 256 * i, 256) for i in range(7)]
NSA = 2                  # number of 128-wide stripes
NSB = 7                  # number of 256-wide stripes

BIGK = float(2 ** 30)

TANH = mybir.ActivationFunctionType.Tanh
MULT = mybir.AluOpType.mult
ADD = mybir.AluOpType.add


def build_bass():
    nc = bacc.Bacc("TRN2", target_bir_lowering=False, debug=False, num_devices=N_CORES)

    xtb = nc.dram_tensor("xtb", [NB, 128, K1, 512], BF16, kind="ExternalInput")
    # L1 ternarize blocks, contiguous per (stripe, kg): [128, 4, cw]
    w1a = nc.dram_tensor("w1a", [NSA, KG1, 128, 4, 128], F32, kind="ExternalInput")
    n1a = nc.dram_tensor("n1a", [NSA, KG1, 128, 4, 128], F32, kind="ExternalInput")
    w1b = nc.dram_tensor("w1b", [NSB, KG1, 128, 4, 256], F32, kind="ExternalInput")
    n1b = nc.dram_tensor("n1b", [NSB, KG1, 128, 4, 256], F32, kind="ExternalInput")
    s1h = nc.dram_tensor("s1h", [128, MT], F32, kind="ExternalInput")
    b1m = nc.dram_tensor("b1m", [128, MT], F32, kind="ExternalInput")
    # L2 ternarize blocks: (kg2, stripe) -> [128, 4, 256]
    w2g = nc.dram_tensor("w2g", [KG2, 4, 128, 4, 256], F32, kind="ExternalInput")
    n2g = nc.dram_tensor("n2g", [KG2, 4, 128, 4, 256], F32, kind="ExternalInput")
    s2d = nc.dram_tensor("s2d", [128, ND], F32, kind="ExternalInput")
    b2d = nc.dram_tensor("b2d", [128, ND], F32, kind="ExternalInput")

    # output: quarter q of block b -> this core's [QCHUNK, 512] slice
    yo = nc.dram_tensor("yo", [NB, NQ, QCHUNK, 512], BF16, kind="ExternalOutput")

    with tile.TileContext(nc) as tc:
        with (
            tc.tile_pool(name="const", bufs=1) as cpool,
            tc.tile_pool(name="dram", bufs=1, space="DRAM") as dpool,
            tc.tile_pool(name="t2w1", bufs=KG1) as t2pool,
            tc.tile_pool(name="t2w2", bufs=1) as t22pool,
            tc.tile_pool(name="stage", bufs=4) as spool,
            tc.tile_pool(name="act8", bufs=2) as apool,
            tc.tile_pool(name="xtn", bufs=2) as xpool,
            tc.tile_pool(name="hblk", bufs=2 * MT) as hpool,
            tc.tile_pool(name="yblk", bufs=4) as ypool,
            tc.tile_pool(name="ps1", bufs=6, space="PSUM") as pspool,
            tc.tile_pool(name="ps2", bufs=2, space="PSUM") as ps2pool,
        ):
            s1_sb = cpool.tile([128, MT], F32, tag="s1")
            b1_sb = cpool.tile([128, MT], F32, tag="b1")
            s2_sb = cpool.tile([128, ND], F32, tag="s2")
            b2_sb = cpool.tile([128, ND], F32, tag="b2")
            nc.scalar.dma_start(s1_sb[:], s1h[:, :])
            nc.scalar.dma_start(b1_sb[:], b1m[:, :])
            nc.scalar.dma_start(s2_sb[:], s2d[:, :])
            nc.scalar.dma_start(b2_sb[:], b2d[:, :])
            kneg = cpool.tile([128, 1], F32, tag="kneg")
            nc.vector.memset(kneg[:], -BIGK)
            kpos = cpool.tile([128, 1], F32, tag="kpos")
            nc.vector.memset(kpos[:], BIGK)

            # per (block, quarter) partial buffers (RS input), bf16
            yq = [[dpool.tile([QROWS, 512], BF16, tag=f"yq{b}{q}",
                              name=f"yq_b{b}q{q}") for q in range(NQ)]
                  for b in range(NB)]
            # RS outputs (collectives cannot write IO tensors directly)
            ro = [[dpool.tile([QCHUNK, 512], BF16, tag=f"ro{b}{q}",
                              name=f"ro_b{b}q{q}") for q in range(NQ)]
                  for b in range(NB)]

            xtn_tiles = {}
            for b in (0, 1):
                xtn_tiles[b] = xpool.tile([128, K1, 512], BF16, tag="xtn",
                                          name=f"xtn{b}")
                nc.scalar.dma_start(xtn_tiles[b][:], xtb[b])

            # ---- ternarize supply ----
            t2g = [t2pool.tile([128, 4, HSH], FP8, tag="t2", name=f"t2g_{kg}")
                   for kg in range(KG1)]
            t22 = t22pool.tile([128, K2, DOUT], FP8, tag="t22")

            def tern_block(dst_ap, w_src, n_src, cw):
                w_t = spool.tile([128, 4, cw], F32, tag="w")
                nc.sync.dma_start(w_t[:], w_src)
                n_t = spool.tile([128, 4, cw], F32, tag="n")
                nc.gpsimd.dma_start(n_t[:], n_src)
                nc.vector.tensor_sub(w_t[:], w_t[:], n_t[:])
                a1 = apool.tile([128, 4, cw], FP8, tag="a1")
                nc.scalar.activation(a1[:], w_t[:], TANH, bias=kneg[:, 0:1], scale=BIGK)
                a2 = apool.tile([128, 4, cw], FP8, tag="a2")
                nc.scalar.activation(a2[:], w_t[:], TANH, bias=kpos[:, 0:1], scale=BIGK)
                nc.vector.tensor_add(dst_ap, a1[:], a2[:])

            def supply_l1_stripe(s):
                c0, cw = STRIPES[s]
                for kg in range(KG1):
                    if cw == 128:
                        w_src, n_src = w1a[s, kg], n1a[s, kg]
                    else:
                        w_src, n_src = w1b[s - NSA, kg], n1b[s - NSA, kg]
                    tern_block(t2g[kg][:, :, c0:c0 + cw], w_src, n_src, cw)

            def supply_l2():
                # stripe-major so each d-pair completes early-to-late
                for s in range(4):
                    for g in range(KG2):
                        tern_block(
                            t22[:, g * 4:(g + 1) * 4, s * 256:(s + 1) * 256],
                            w2g[g, s], n2g[g, s], 256,
                        )

            for s in range(len(STRIPES)):
                supply_l1_stripe(s)
            supply_l2()
            # late x blocks: issued on sync queue after all weight supply
            for b in (2, 3):
                xtn_tiles[b] = xpool.tile([128, K1, 512], BF16, tag="xtn",
                                          name=f"xtn{b}")
                nc.sync.dma_start(xtn_tiles[b][:], xtb[b])

            # ---- compute ----
            h_sets = {0: [], 1: [], 2: [], 3: []}

            def layer1_chain(b, m):
                xtn = xtn_tiles[b]
                ps = pspool.tile([128, 512], F32, tag="ps")
                for k in range(K1):
                    nc.tensor.matmul(
                        ps[:],
                        t2g[k // 4][:, k % 4, m * 128:(m + 1) * 128],
                        xtn[:, k, :],
                        start=(k == 0), stop=(k == K1 - 1))
                h_m = hpool.tile([128, 512], BF16, tag="h")
                nc.scalar.activation(
                    h_m[:], ps[:], TANH,
                    bias=b1_sb[:, m:m + 1], scale=s1_sb[:, m:m + 1],
                )
                h_sets[b].append(h_m)

            def layer2_block(b):
                for d in range(ND):
                    p = ps2pool.tile([128, 512], F32, tag="ps2")
                    for k2 in range(K2):
                        nc.tensor.matmul(p[:], t22[:, k2, d * 128:(d + 1) * 128],
                                         h_sets[b][k2][:],
                                         start=(k2 == 0), stop=(k2 == K2 - 1))
                    y_sb = ypool.tile([128, 512], BF16, tag="y")
                    nc.vector.tensor_scalar(
                        y_sb[:], p[:], s2_sb[:, d:d + 1], b2_sb[:, d:d + 1],
                        MULT, ADD,
                    )
                    q, half = d // 2, d % 2
                    nc.sync.dma_start(
                        yq[b][q][half * 128:(half + 1) * 128, :], y_sb[:],
                    )
                    if half == 1:
                        nc.gpsimd.collective_compute(
                            "ReduceScatter",
                            mybir.AluOpType.add,
                            replica_groups=[list(range(N_CORES))],
                            ins=[yq[b][q].opt()],
                            outs=[ro[b][q].opt()],
                        )
                        nc.sync.dma_start(yo[b, q], ro[b][q][:])

            # blocks 0/1: per-m-tile interleave tracking the supply stripes
            for m in range(MT):
                layer1_chain(0, m)
                layer1_chain(1, m)
            layer2_block(0)
            layer2_block(1)

            # blocks 2/3 from resident weights
            for b in (2, 3):
                for m in range(MT):
                    layer1_chain(b, m)
                layer2_block(b)

    nc.compile()
    return nc


_NC_CACHE = {}


def _get_nc():
    if "nc" not in _NC_CACHE:
        _NC_CACHE["nc"] = build_bass()
    return _NC_CACHE["nc"]


def _tile_l1(w):
    """[DIN, HSH] -> (w1a [NSA,KG1,128,4,128], w1b [NSB,KG1,128,4,256])"""
    # block (s, kg) holds rows (kg*4+j)*128 + p, cols c0 + c
    wk = w.reshape(KG1, 4, 128, HSH)          # [kg, j, p, col]
    a = np.empty((NSA, KG1, 128, 4, 128), dtype=np.float32)
    b = np.empty((NSB, KG1, 128, 4, 256), dtype=np.float32)
    for s, (c0, cw) in enumerate(STRIPES):
        blk = wk[:, :, :, c0:c0 + cw].transpose(0, 2, 1, 3)  # [kg, p, j, cw]
        if s < NSA:
            a[s] = blk
        else:
            b[s - NSA] = blk
    return np.ascontiguousarray(a), np.ascontiguousarray(b)


def _tile_l2(w):
    """[HSH, DOUT] -> [KG2, 4, 128, 4, 256]"""
    wk = w.reshape(KG2, 4, 128, DOUT)         # [g, j, p, col]
    out = np.empty((KG2, 4, 128, 4, 256), dtype=np.float32)
    for s in range(4):
        out[:, s] = wk[:, :, :, s * 256:(s + 1) * 256].transpose(0, 2, 1, 3)
    return np.ascontiguousarray(out)


def _make_in_maps(x, w1, s1, b1, w2, s2, b2, noise1, noise2):
    x = np.asarray(x, dtype=np.float32)
    w1 = np.asarray(w1, dtype=np.float32)
    s1 = np.asarray(s1, dtype=np.float32)
    b1 = np.asarray(b1, dtype=np.float32)
    w2 = np.asarray(w2, dtype=np.float32)
    s2 = np.asarray(s2, dtype=np.float32)
    b2 = np.asarray(b2, dtype=np.float32)
    noise1 = np.asarray(noise1, dtype=np.float32)
    noise2 = np.asarray(noise2, dtype=np.float32)

    xT = x.T.astype(NPBF16)
    xtb = np.ascontiguousarray(xT.reshape(K1, 128, NB, 512).transpose(2, 1, 0, 3))

    in_maps = []
    for c in range(N_CORES):
        hs = slice(c * HSH, (c + 1) * HSH)
        w1a, w1b = _tile_l1(np.ascontiguousarray(w1[:, hs]))
        n1a, n1b = _tile_l1(np.ascontiguousarray(noise1[:, hs]))
        # s2/b2 per d-tile column; x2 tern factor folded as 0.5; b2 on core 0
        s2m = np.ascontiguousarray((0.5 * s2).reshape(ND, 128).T)
        b2m = np.ascontiguousarray(b2.reshape(ND, 128).T) if c == 0 else \
            np.zeros((128, ND), dtype=np.float32)
        in_maps.append({
            "xtb": xtb,
            "w1a": w1a, "w1b": w1b,
            "n1a": n1a, "n1b": n1b,
            "s1h": np.ascontiguousarray((0.5 * s1[hs]).reshape(MT, 128).T),
            "b1m": np.ascontiguousarray(b1[hs].reshape(MT, 128).T),
            "w2g": _tile_l2(np.ascontiguousarray(w2[hs, :])),
            "n2g": _tile_l2(np.ascontiguousarray(noise2[hs, :])),
            "s2d": s2m,
            "b2d": b2m,
        })
    return in_maps


def kernel(x, w1, s1, b1, w2, s2, b2, noise1, noise2, _bench_out=None):
    """Full-input, full-output entry point. Shards across 8 NeuronCores."""
    nc = _get_nc()
    in_maps = _make_in_maps(x, w1, s1, b1, w2, s2, b2, noise1, noise2)
    res = run_bass_kernel_spmd(nc, in_maps, core_ids=list(range(N_CORES)))
    if _bench_out is not None:
        _bench_out.append(res)
    yT = np.empty((DOUT, B), dtype=np.float32)
    for c in range(N_CORES):
        out_c = np.asarray(res.results[c]["yo"]).astype(np.float32)
        for q in range(NQ):
            r0 = q * QROWS + c * QCHUNK
            for b in range(NB):
                yT[r0:r0 + QCHUNK, b * 512:(b + 1) * 512] = out_c[b, q]
    return np.ascontiguousarray(yT.T).astype(np.float32)


if __name__ == "__main__":
    nc = build_bass()
    print("built OK")


# revision 14
# speedup vs baseline: 1.0259x; 1.0259x over previous
"""Trainium2 Bass kernel for nn_Encoder (dense MLP with stochastic ternarization).

y = tanh(x @ (s1*T(w1,n1)) + b1) @ (s2*T(w2,n2)) + b2,  T(w,n) = (w-n>1) - (w-n<-1)

Sharding: tensor-parallel over the 16384 hidden dim across 8 cores. Each core
gets a 2048-wide hidden shard of w1/noise1/s1/b1 (column-sharded) and the
matching 2048-row shard of w2/noise2; x is replicated (host pre-transposed to
bf16, tiled per 512-batch block).

v2 (this file) vs the 648us baseline:
- w/n supply is host-packed into contiguous 0.5 MiB ternarize blocks in exact
  consumption order (two 128-wide stripes first so the first m-chains unblock
  ~15us earlier, then 256-wide stripes). w blocks issue on the sync queue,
  n blocks on the gpsimd queue; staging is 4-deep.
- s2/b2 are applied at PSUM evacuation (s2 scale on every core's partial, b2
  added on core 0 only), partials stored bf16, and each (block, half) of
  yT goes through its own bf16 ReduceScatter(add) into a DRAM scratch that
  is DMA-copied to the output. No post-collective compute; the tail RS is
  512 KiB bf16 instead of 1 MiB fp32.
- Two 1 KiB pre-sync collectives during L1(b3) absorb accumulated inter-core
  skew so the final block's ReduceScatters don't pay a ~40us peer-wait.
- The w-n subtract runs on GpSimd, the tanh-pair on ACT, the fp8 add on DVE:
  three-engine ternarize pipeline with ~40% headroom over PE consumption.
- L1 consumption is per-m-tile interleaved across batch blocks 0/1 so PE
  consumption tracks the DMA-paced ternarize supply.

Ternarization: q = w - noise (DVE), tanh(2^30*(q-1)) + tanh(2^30*(q+1)) (ACT)
== (q>1)-(q<-1) doubled; the factor 2 is folded into s1/s2 on the host.
"""

import sys

for _p in ("/opt/trn_rl_repo",):
    if _p not in sys.path:
        sys.path.insert(0, _p)

import numpy as np
import ml_dtypes

import concourse.bass as bass
import concourse.bacc as bacc
import concourse.mybir as mybir
import concourse.tile as tile
from concourse.bass_utils import run_bass_kernel_spmd

BF16 = mybir.dt.bfloat16
F32 = mybir.dt.float32
FP8 = mybir.dt.float8e4
NPBF16 = ml_dtypes.bfloat16

N_CORES = 8
B = 2048
DIN = 3072
DHID = 16384
DOUT = 1024
HSH = DHID // N_CORES   # 2048
DSH = DOUT // N_CORES   # 128

K1 = DIN // 128          # 24 contraction tiles, layer 1
KG1 = K1 // 4            # 6 groups of 4 k-tiles
K2 = HSH // 128          # 16 contraction tiles, layer 2
KG2 = K2 // 4            # 4 groups of 4 k2-tiles
NB = B // 512            # 4 batch blocks
MT = HSH // 128          # 16 hidden m-tiles
ND = DOUT // 128         # 8 dout tiles
NH = 2                   # output halves per block (4 d-tiles each)
QROWS = DOUT // NH       # 512 dout rows per half
QCHUNK = QROWS // N_CORES  # 64 rows per core per half

# L1 supply stripes: (col0, width); first two are 128-wide for fast start
STRIPES = [(0, 128), (128, 128)] + [(256 + 256 * i, 256) for i in range(7)]
NSA = 2                  # number of 128-wide stripes
NSB = 7                  # number of 256-wide stripes

BIGK = float(2 ** 30)

TANH = mybir.ActivationFunctionType.Tanh
MULT = mybir.AluOpType.mult
ADD = mybir.AluOpType.add


def build_bass():
    nc = bacc.Bacc("TRN2", target_bir_lowering=False, debug=False, num_devices=N_CORES)

    xtb = nc.dram_tensor("xtb", [NB, 128, K1, 512], BF16, kind="ExternalInput")
    # L1 ternarize blocks, contiguous per (stripe, kg): [128, 4, cw]
    w1a = nc.dram_tensor("w1a", [NSA, KG1, 128, 4, 128], F32, kind="ExternalInput")
    n1a = nc.dram_tensor("n1a", [NSA, KG1, 128, 4, 128], F32, kind="ExternalInput")
    w1b = nc.dram_tensor("w1b", [NSB, KG1, 128, 4, 256], F32, kind="ExternalInput")
    n1b = nc.dram_tensor("n1b", [NSB, KG1, 128, 4, 256], F32, kind="ExternalInput")
    s1h = nc.dram_tensor("s1h", [128, MT], F32, kind="ExternalInput")
    b1m = nc.dram_tensor("b1m", [128, MT], F32, kind="ExternalInput")
    # L2 ternarize blocks: (kg2, stripe) -> [128, 4, 256]
    w2g = nc.dram_tensor("w2g", [KG2, 4, 128, 4, 256], F32, kind="ExternalInput")
    n2g = nc.dram_tensor("n2g", [KG2, 4, 128, 4, 256], F32, kind="ExternalInput")
    s2d = nc.dram_tensor("s2d", [128, ND], F32, kind="ExternalInput")
    b2d = nc.dram_tensor("b2d", [128, ND], F32, kind="ExternalInput")

    # output: half h of block b -> this core's [QCHUNK, 512] slice
    yo = nc.dram_tensor("yo", [NB, NH, QCHUNK, 512], BF16, kind="ExternalOutput")

    with tile.TileContext(nc) as tc:
        with (
            tc.tile_pool(name="const", bufs=1) as cpool,
            tc.tile_pool(name="dram", bufs=1, space="DRAM") as dpool,
            tc.tile_pool(name="t2w1", bufs=KG1) as t2pool,
            tc.tile_pool(name="t2w2", bufs=1) as t22pool,
            tc.tile_pool(name="stage", bufs=4) as spool,
            tc.tile_pool(name="act8", bufs=4) as apool,
            tc.tile_pool(name="xtn", bufs=2) as xpool,
            tc.tile_pool(name="hblk", bufs=2 * MT) as hpool,
            tc.tile_pool(name="yblk", bufs=4) as ypool,
            tc.tile_pool(name="ps1", bufs=6, space="PSUM") as pspool,
            tc.tile_pool(name="ps2", bufs=2, space="PSUM") as ps2pool,
        ):
            s1_sb = cpool.tile([128, MT], F32, tag="s1")
            b1_sb = cpool.tile([128, MT], F32, tag="b1")
            s2_sb = cpool.tile([128, ND], F32, tag="s2")
            b2_sb = cpool.tile([128, ND], F32, tag="b2")
            nc.scalar.dma_start(s1_sb[:], s1h[:, :])
            nc.scalar.dma_start(b1_sb[:], b1m[:, :])
            nc.scalar.dma_start(s2_sb[:], s2d[:, :])
            nc.scalar.dma_start(b2_sb[:], b2d[:, :])
            kneg = cpool.tile([128, 1], F32, tag="kneg")
            nc.vector.memset(kneg[:], -BIGK)
            kpos = cpool.tile([128, 1], F32, tag="kpos")
            nc.vector.memset(kpos[:], BIGK)

            # per (block, half) partial buffers (RS input), bf16
            yq = [[dpool.tile([QROWS, 512], BF16, tag=f"yq{b}{q}",
                              name=f"yq_b{b}q{q}") for q in range(NH)]
                  for b in range(NB)]
            # RS outputs (collectives cannot write IO tensors directly)
            ro = [[dpool.tile([QCHUNK, 512], BF16, tag=f"ro{b}{q}",
                              name=f"ro_b{b}q{q}") for q in range(NH)]
                  for b in range(NB)]
            # tiny pre-sync collective buffers (absorb inter-core skew
            # before the last block's RS burst)
            dsrc = [dpool.tile([8, 64], BF16, tag=f"dsrc{i}",
                               name=f"dsrc{i}") for i in range(2)]
            ddst = [dpool.tile([1, 64], BF16, tag=f"ddst{i}",
                               name=f"ddst{i}") for i in range(2)]

            xtn_tiles = {}
            for b in (0, 1):
                xtn_tiles[b] = xpool.tile([128, K1, 512], BF16, tag="xtn",
                                          name=f"xtn{b}")
                nc.scalar.dma_start(xtn_tiles[b][:], xtb[b])

            # ---- ternarize supply ----
            t2g = [t2pool.tile([128, 4, HSH], FP8, tag="t2", name=f"t2g_{kg}")
                   for kg in range(KG1)]
            t22 = t22pool.tile([128, K2, DOUT], FP8, tag="t22")

            def tern_block(dst_ap, w_src, n_src, cw):
                w_t = spool.tile([128, 4, cw], F32, tag="w")
                nc.sync.dma_start(w_t[:], w_src)
                n_t = spool.tile([128, 4, cw], F32, tag="n")
                nc.gpsimd.dma_start(n_t[:], n_src)
                nc.gpsimd.tensor_sub(w_t[:], w_t[:], n_t[:])
                a1 = apool.tile([128, 4, cw], FP8, tag="a1")
                nc.scalar.activation(a1[:], w_t[:], TANH, bias=kneg[:, 0:1], scale=BIGK)
                a2 = apool.tile([128, 4, cw], FP8, tag="a2")
                nc.scalar.activation(a2[:], w_t[:], TANH, bias=kpos[:, 0:1], scale=BIGK)
                nc.vector.tensor_add(dst_ap, a1[:], a2[:])

            def presync(i, dep_tile):
                # tiny store (creates the trigger-time dependency), then a
                # 1 KiB ReduceScatter purely to sync the 8 cores' CC streams
                nc.sync.dma_start(dsrc[i][:], dep_tile[0:8, 0:64])
                nc.gpsimd.collective_compute(
                    "ReduceScatter",
                    mybir.AluOpType.add,
                    replica_groups=[list(range(N_CORES))],
                    ins=[dsrc[i].opt()],
                    outs=[ddst[i].opt()],
                )

            def supply_l1_stripe(s):
                c0, cw = STRIPES[s]
                for kg in range(KG1):
                    if cw == 128:
                        w_src, n_src = w1a[s, kg], n1a[s, kg]
                    else:
                        w_src, n_src = w1b[s - NSA, kg], n1b[s - NSA, kg]
                    tern_block(t2g[kg][:, :, c0:c0 + cw], w_src, n_src, cw)

            def supply_l2():
                # stripe-major so each d-pair completes early-to-late
                for s in range(4):
                    for g in range(KG2):
                        tern_block(
                            t22[:, g * 4:(g + 1) * 4, s * 256:(s + 1) * 256],
                            w2g[g, s], n2g[g, s], 256,
                        )

            for s in range(len(STRIPES)):
                supply_l1_stripe(s)
            supply_l2()
            # late x blocks: issued on sync queue after all weight supply
            for b in (2, 3):
                xtn_tiles[b] = xpool.tile([128, K1, 512], BF16, tag="xtn",
                                          name=f"xtn{b}")
                nc.sync.dma_start(xtn_tiles[b][:], xtb[b])

            # ---- compute ----
            h_sets = {0: [], 1: [], 2: [], 3: []}

            def layer1_chain(b, m):
                xtn = xtn_tiles[b]
                ps = pspool.tile([128, 512], F32, tag="ps")
                for k in range(K1):
                    nc.tensor.matmul(
                        ps[:],
                        t2g[k // 4][:, k % 4, m * 128:(m + 1) * 128],
                        xtn[:, k, :],
                        start=(k == 0), stop=(k == K1 - 1))
                h_m = hpool.tile([128, 512], BF16, tag="h")
                nc.scalar.activation(
                    h_m[:], ps[:], TANH,
                    bias=b1_sb[:, m:m + 1], scale=s1_sb[:, m:m + 1],
                )
                h_sets[b].append(h_m)

            def layer2_block(b):
                for d in range(ND):
                    p = ps2pool.tile([128, 512], F32, tag="ps2")
                    for k2 in range(K2):
                        nc.tensor.matmul(p[:], t22[:, k2, d * 128:(d + 1) * 128],
                                         h_sets[b][k2][:],
                                         start=(k2 == 0), stop=(k2 == K2 - 1))
                    y_sb = ypool.tile([128, 512], BF16, tag="y")
                    nc.vector.tensor_scalar(
                        y_sb[:], p[:], s2_sb[:, d:d + 1], b2_sb[:, d:d + 1],
                        MULT, ADD,
                    )
                    q, part = d // 4, d % 4
                    nc.sync.dma_start(
                        yq[b][q][part * 128:(part + 1) * 128, :], y_sb[:],
                    )
                    if part == 3:
                        nc.gpsimd.collective_compute(
                            "ReduceScatter",
                            mybir.AluOpType.add,
                            replica_groups=[list(range(N_CORES))],
                            ins=[yq[b][q].opt()],
                            outs=[ro[b][q].opt()],
                        )
                        nc.sync.dma_start(yo[b, q], ro[b][q][:])

            # blocks 0/1: per-m-tile interleave tracking the supply stripes
            for m in range(MT):
                layer1_chain(0, m)
                layer1_chain(1, m)
            layer2_block(0)
            layer2_block(1)

            # blocks 2/3 from resident weights
            for b in (2, 3):
                for m in range(MT):
                    layer1_chain(b, m)
                    if b == 3 and m in (0, 12):
                        presync(0 if m == 0 else 1, h_sets[3][-1])
                layer2_block(b)

    nc.compile()
    return nc


_NC_CACHE = {}


def _get_nc():
    if "nc" not in _NC_CACHE:
        _NC_CACHE["nc"] = build_bass()
    return _NC_CACHE["nc"]


def _tile_l1(w):
    """[DIN, HSH] -> (w1a [NSA,KG1,128,4,128], w1b [NSB,KG1,128,4,256])"""
    # block (s, kg) holds rows (kg*4+j)*128 + p, cols c0 + c
    wk = w.reshape(KG1, 4, 128, HSH)          # [kg, j, p, col]
    a = np.empty((NSA, KG1, 128, 4, 128), dtype=np.float32)
    b = np.empty((NSB, KG1, 128, 4, 256), dtype=np.float32)
    for s, (c0, cw) in enumerate(STRIPES):
        blk = wk[:, :, :, c0:c0 + cw].transpose(0, 2, 1, 3)  # [kg, p, j, cw]
        if s < NSA:
            a[s] = blk
        else:
            b[s - NSA] = blk
    return np.ascontiguousarray(a), np.ascontiguousarray(b)


def _tile_l2(w):
    """[HSH, DOUT] -> [KG2, 4, 128, 4, 256]"""
    wk = w.reshape(KG2, 4, 128, DOUT)         # [g, j, p, col]
    out = np.empty((KG2, 4, 128, 4, 256), dtype=np.float32)
    for s in range(4):
        out[:, s] = wk[:, :, :, s * 256:(s + 1) * 256].transpose(0, 2, 1, 3)
    return np.ascontiguousarray(out)


def _make_in_maps(x, w1, s1, b1, w2, s2, b2, noise1, noise2):
    x = np.asarray(x, dtype=np.float32)
    w1 = np.asarray(w1, dtype=np.float32)
    s1 = np.asarray(s1, dtype=np.float32)
    b1 = np.asarray(b1, dtype=np.float32)
    w2 = np.asarray(w2, dtype=np.float32)
    s2 = np.asarray(s2, dtype=np.float32)
    b2 = np.asarray(b2, dtype=np.float32)
    noise1 = np.asarray(noise1, dtype=np.float32)
    noise2 = np.asarray(noise2, dtype=np.float32)

    xT = x.T.astype(NPBF16)
    xtb = np.ascontiguousarray(xT.reshape(K1, 128, NB, 512).transpose(2, 1, 0, 3))

    in_maps = []
    for c in range(N_CORES):
        hs = slice(c * HSH, (c + 1) * HSH)
        w1a, w1b = _tile_l1(np.ascontiguousarray(w1[:, hs]))
        n1a, n1b = _tile_l1(np.ascontiguousarray(noise1[:, hs]))
        # s2/b2 per d-tile column; x2 tern factor folded as 0.5; b2 on core 0
        s2m = np.ascontiguousarray((0.5 * s2).reshape(ND, 128).T)
        b2m = np.ascontiguousarray(b2.reshape(ND, 128).T) if c == 0 else \
            np.zeros((128, ND), dtype=np.float32)
        in_maps.append({
            "xtb": xtb,
            "w1a": w1a, "w1b": w1b,
            "n1a": n1a, "n1b": n1b,
            "s1h": np.ascontiguousarray((0.5 * s1[hs]).reshape(MT, 128).T),
            "b1m": np.ascontiguousarray(b1[hs].reshape(MT, 128).T),
            "w2g": _tile_l2(np.ascontiguousarray(w2[hs, :])),
            "n2g": _tile_l2(np.ascontiguousarray(noise2[hs, :])),
            "s2d": s2m,
            "b2d": b2m,
        })
    return in_maps


def kernel(x, w1, s1, b1, w2, s2, b2, noise1, noise2, _bench_out=None):
    """Full-input, full-output entry point. Shards across 8 NeuronCores."""
    nc = _get_nc()
    in_maps = _make_in_maps(x, w1, s1, b1, w2, s2, b2, noise1, noise2)
    res = run_bass_kernel_spmd(nc, in_maps, core_ids=list(range(N_CORES)))
    if _bench_out is not None:
        _bench_out.append(res)
    yT = np.empty((DOUT, B), dtype=np.float32)
    for c in range(N_CORES):
        out_c = np.asarray(res.results[c]["yo"]).astype(np.float32)
        for q in range(NH):
            r0 = q * QROWS + c * QCHUNK
            for b in range(NB):
                yT[r0:r0 + QCHUNK, b * 512:(b + 1) * 512] = out_c[b, q]
    return np.ascontiguousarray(yT.T).astype(np.float32)


if __name__ == "__main__":
    nc = build_bass()
    print("built OK")
